# revision 9
# baseline (speedup 1.0000x reference)
"""Trainium2 Bass kernel for the span-extraction (start/end) cross-entropy loss.

    loss = (1/(2B)) * sum_b [ (LSE_s[b] - s[b, sp_b]) + (LSE_e[b] - e[b, ep_b]) ]

Distribution: data-parallel over the batch axis across 8 NeuronCores (32 rows
per core per tensor).  The kernel is memory-bound; all logits are staged to
the device as fp8-e4m3 (1 B/elem; the 2e-2 rel-err gate leaves orders of
magnitude of headroom — measured end-to-end error is ~5e-5).  Three engines
split the exp+sum work per tensor:

  * ACT share (A cols/partition, row-quarter-major layout): fused exact
    exp + accumulate at 1 elem/cycle/lane.  Two ops per tensor (one small
    starter, one large) to amortize the ~570ns per-op fixed cost (352cyc
    ACTIVATE startup + 279ns accumulator read).
  * DVE share (V cols/partition, TRANSPOSED layout: each SBUF column holds
    128 elements of one batch row): tensor_scalar ops compute
    round(A*x + B) into int16 (Schraudolph: the int16 bit patterns ARE
    bf16(exp(x)); A = 128/ln2, B calibrated so E[schr(x)] = E[exp(x)] on
    N(0,1)).  Runs at 2 elem/cycle/lane (fp8 single-src 2x_2P mode).
  * PE (otherwise idle) reduces the bf16-bitcast int16 tiles over the
    partition axis: all-ones [128,128] matmuls accumulate 256-column
    segments into PSUM, psum[:, gg*32 + r] += sum_p shr[p, seg*256 +
    gg*32 + r] (every output row identical; M=128 keeps all four PE
    column groups engaged — M=1 measured ~2.5x slower per column).  This
    replaces the baseline's second DVE pass entirely.  PE p-states:
    0.65/1.2 GHz until ~3us of continuous activity — keep the stream dense
    and the final chunk small.

DMA: 5 chunks per tensor (10 DMAs; the 9th/10th reuse the completion-sem
lanes of the 1st/2nd, which finish early in the stream, so their dispatch
barely stalls).  s-chunks ride the sync HWDGE ring (Q1), e-chunks the
scalar ring (Q10), equal byte loads.  Queue order interleaves the DVE
windows around the single pure-ACT chunk so both engines' data arrives
roughly when consumed.  The 512 target logits are gathered on the host
from the fp32 originals; the host sums the partials, takes log, and
combines in fp64.
"""

import numpy as np
import ml_dtypes

from contextlib import ExitStack

import concourse.bass as bass
import concourse.bacc as bacc
import concourse.tile as tile
from concourse import mybir
from concourse.bass_utils import run_bass_kernel_spmd

B, S = 256, 32768
N_CORES = 8
ROWS = B // N_CORES          # 32 batch rows per core
QUARTERS = 4                 # ACT share: each row split across 4 partitions
P = ROWS * QUARTERS          # 128 partitions
SEG = S // QUARTERS          # 8192 elements per partition-quarter
LINE_B = SEG                 # all-fp8: 8192 bytes per partition per tensor

# Line layout = queue order.  Each window is ("a", width) or ("v", width);
# w1 carries both a starter ACT part and the first DVE part.
WINDOWS = [("av", 512, 768), ("v", 0, 1792), ("a", 2304, 0),
           ("v", 0, 1792), ("v", 0, 1024)]
A_COLS = sum(wa for _, wa, _ in WINDOWS)      # 2816
V_COLS = sum(wv for _, _, wv in WINDOWS)      # 5376
G = V_COLS // 32                              # 168 groups of 128 per row
SEGS = V_COLS // 256                          # 21 matmul segments
assert all(wv % 256 == 0 for _, _, wv in WINDOWS)
assert A_COLS + V_COLS == LINE_B

CWIN = []                     # (lo, hi) line windows
_off = 0
for _, wa, wv in WINDOWS:
    CWIN.append((_off, _off + wa + wv))
    _off += wa + wv
assert _off == LINE_B
VOFF = []                     # shr column offset per window
_v = 0
for _, _, wv in WINDOWS:
    VOFF.append(_v)
    _v += wv

# Schraudolph constants: schr(x) = bitcast_bf16(int16(A*x + B)), with the
# f32->i16 conversion rounding to nearest (verified on HW: rel err ~1e-6).
A_SCHR = 128.0 / float(np.log(2.0))
B_SCHR = 16256.0 - 7.367385

_CACHE = {}

LAST_RESULT = None           # BassKernelResults of the most recent run


def _build():
    f32 = mybir.dt.float32
    bf16 = mybir.dt.bfloat16
    f8 = mybir.dt.float8e4
    u8 = mybir.dt.uint8
    i16 = mybir.dt.int16
    nc = bacc.Bacc(
        "TRN2", target_bir_lowering=False, debug=False, num_devices=N_CORES
    )
    x_in = {
        nm: nc.dram_tensor(f"x_{nm}", [P, LINE_B], u8, kind="ExternalInput").ap()
        for nm in ("s", "e")
    }
    psa_out = nc.dram_tensor("ps_a", [P, 4], f32, kind="ExternalOutput").ap()
    pe_out_d = nc.dram_tensor("pe_o", [1, 512], f32, kind="ExternalOutput").ap()

    w1 = WINDOWS[0][1]                     # op1 cols (512)
    w2 = WINDOWS[2][1]                     # op2 cols (2304)
    a2lo = CWIN[2][0]

    with tile.TileContext(nc) as tc, ExitStack() as ctx:
        data_pool = ctx.enter_context(tc.tile_pool(name="data", bufs=1))
        small_pool = ctx.enter_context(tc.tile_pool(name="small", bufs=1))
        psum_pool = ctx.enter_context(
            tc.tile_pool(name="psum", bufs=1, space="PSUM")
        )

        xbuf = {
            nm: data_pool.tile([P, LINE_B], u8, name=f"x_{nm}", tag=f"x_{nm}")
            for nm in ("s", "e")
        }
        shr = {
            nm: data_pool.tile([P, V_COLS], i16, name=f"sh_{nm}", tag=f"sh_{nm}")
            for nm in ("s", "e")
        }
        scr = {
            nm: data_pool.tile([P, w2], bf16, name=f"sc_{nm}", tag=f"sc_{nm}")
            for nm in ("s", "e")
        }
        acc_a = small_pool.tile([P, 4], f32, tag="acc_a")
        pe_sb = small_pool.tile([1, 512], f32, tag="pe_sb")
        ones = small_pool.tile([P, P], bf16, tag="ones")
        psum = {
            nm: psum_pool.tile([P, 512], f32, name=f"pm_{nm}", tag=f"pm_{nm}")
            for nm in ("s", "e")
        }

        # all-ones weights for the PE partition-reduction (M=128 engages all
        # four PE column groups); runs during the preamble.
        nc.vector.memset(ones[:], 1.0)

        # Data DMAs: s-chunks on the sync ring, e-chunks on the scalar
        # ring, emission interleaved so the 9th/10th DMA reuse the sem
        # lanes of the earliest-completing chunks (c1 of each tensor).
        # Both sem-reusing dispatches (s_c5, e_c5) go via the otherwise-
        # idle sync engine: a reused lane makes the dispatch WAIT for the
        # predecessor's completion, which on the scalar engine would
        # delay the first ACTIVATE (~1.2us measured).
        for lo, hi in CWIN[:-1]:
            nc.sync.dma_start(xbuf["s"][:, lo:hi], x_in["s"][:, lo:hi])
            nc.scalar.dma_start(xbuf["e"][:, lo:hi], x_in["e"][:, lo:hi])
        lo, hi = CWIN[-1]
        nc.sync.dma_start(xbuf["s"][:, lo:hi], x_in["s"][:, lo:hi])
        nc.sync.dma_start(xbuf["e"][:, lo:hi], x_in["e"][:, lo:hi])

        # ACT: exact exp + fused accumulate; op1 = w1's starter ACT part,
        # op2 = the pure-ACT window.
        acol = {("s", 0): 0, ("e", 0): 1, ("s", 1): 2, ("e", 1): 3}
        for nm in ("s", "e"):
            va = xbuf[nm].bitcast(f8)
            nc.scalar.activation(
                scr[nm][:, :w1],
                va[:, 0:w1],
                mybir.ActivationFunctionType.Exp,
                accum_out=acc_a[:, acol[nm, 0] : acol[nm, 0] + 1],
            )
            nc.scalar.activation(
                scr[nm][:, :w2],
                va[:, a2lo : a2lo + w2],
                mybir.ActivationFunctionType.Exp,
                accum_out=acc_a[:, acol[nm, 1] : acol[nm, 1] + 1],
            )

        # DVE pass 1 (one op per v-window, s/e interleaved) and PE
        # reduction (256-col segments accumulated into PSUM, emission in
        # arrival order; copies right after each tensor's last segment).
        vwins = [(ci, wa, wv) for ci, (_, wa, wv) in enumerate(WINDOWS) if wv]
        for k, (ci, wa, wv) in enumerate(vwins):
            lo = CWIN[ci][0] + wa
            for nm in ("s", "e"):
                va = xbuf[nm].bitcast(f8)
                nc.vector.tensor_scalar(
                    shr[nm][:, VOFF[ci] : VOFF[ci] + wv],
                    va[:, lo : lo + wv],
                    A_SCHR,
                    B_SCHR,
                    mybir.AluOpType.mult,
                    mybir.AluOpType.add,
                )
            s0 = VOFF[ci] // 256
            for nm in ("s", "e"):
                sv = shr[nm].bitcast(bf16)
                for sg in range(s0, s0 + wv // 256):
                    nc.tensor.matmul(
                        psum[nm][:, 0:256],
                        ones[:],
                        sv[:, sg * 256 : (sg + 1) * 256],
                        start=(sg == 0),
                        stop=(sg == SEGS - 1),
                    )
                if k == len(vwins) - 1:
                    dst = pe_sb[0:1, 0:256] if nm == "s" else pe_sb[0:1, 256:512]
                    nc.vector.tensor_copy(dst, psum[nm][0:1, 0:256])

        # Outputs: psa as soon as the ACT reads finish (scalar ring), the
        # PE row after the copies (sync ring).
        nc.scalar.dma_start(psa_out, acc_a[:])
        nc.sync.dma_start(pe_out_d, pe_sb[:])
    nc.compile()
    return nc


def _get_nc():
    if "nc" not in _CACHE:
        _CACHE["nc"] = _build()
    return _CACHE["nc"]


def _stage(x2):
    """[B, S] f32 -> per-core [128, 8192] u8 lines in the windowed layout.

    Returns [N_CORES, 128, LINE_B] u8.  ACT parts (row-quarter-major:
    partition r*4+q holds fp8 of that quarter's next wa cols) and DVE
    parts (transposed: column g*32 + r holds fp8 of 128 elements of row
    r; columns assigned to windows sequentially)."""
    f8np = mybir.dt.np(mybir.dt.float8e4)
    x3 = x2.reshape(B, QUARTERS, SEG)
    out = np.empty((N_CORES, P, LINE_B), np.uint8)
    for i in range(N_CORES):
        rs = slice(i * ROWS, (i + 1) * ROWS)
        act = np.ascontiguousarray(x3[rs, :, :A_COLS]).astype(f8np)
        act = act.reshape(P, A_COLS).view(np.uint8)
        dve = np.ascontiguousarray(x3[rs, :, A_COLS:]).astype(f8np)
        # [32, 4*(SEG-A)] -> [32, G, 128] -> [128, G, 32] -> [128, V]
        dve = dve.reshape(ROWS, G, 128).transpose(2, 1, 0)
        dve = np.ascontiguousarray(dve).reshape(P, V_COLS).view(np.uint8)
        aoff = voff = 0
        for ci, (_, wa, wv) in enumerate(WINDOWS):
            lo = CWIN[ci][0]
            if wa:
                out[i, :, lo : lo + wa] = act[:, aoff : aoff + wa]
                aoff += wa
            if wv:
                out[i, :, lo + wa : lo + wa + wv] = dve[:, voff : voff + wv]
                voff += wv
    return out


def kernel(start_logits, end_logits, start_positions, end_positions):
    global LAST_RESULT
    s2 = np.ascontiguousarray(np.asarray(start_logits, dtype=np.float32).reshape(B, S))
    e2 = np.ascontiguousarray(np.asarray(end_logits, dtype=np.float32).reshape(B, S))
    sp = np.asarray(start_positions).astype(np.int64)
    ep = np.asarray(end_positions).astype(np.int64)

    s_st = _stage(s2)
    e_st = _stage(e2)

    in_maps = [
        {"x_s": s_st[i], "x_e": e_st[i]} for i in range(N_CORES)
    ]

    nc = _get_nc()
    res = run_bass_kernel_spmd(nc, in_maps, list(range(N_CORES)))
    LAST_RESULT = res

    total = 0.0
    rr = np.arange(ROWS)
    for i in range(N_CORES):
        rs = slice(i * ROWS, (i + 1) * ROWS)
        r = res.results[i]
        pa = np.asarray(r["ps_a"], np.float64)     # [P, 4]: (s1, e1, s2, e2)
        pe = np.asarray(r["pe_o"], np.float64)[0]  # [512]: s 0:256, e 256:512
        pa4 = pa.reshape(ROWS, QUARTERS, 4).sum(axis=1)   # [ROWS, 4]
        act_s = pa4[:, 0] + pa4[:, 2]
        act_e = pa4[:, 1] + pa4[:, 3]
        dve_s = pe[:256].reshape(8, ROWS).sum(axis=0)
        dve_e = pe[256:].reshape(8, ROWS).sum(axis=0)
        lse_s = np.log(act_s + dve_s)
        lse_e = np.log(act_e + dve_e)
        g_s = s2[rs][rr, sp[rs]].astype(np.float64)
        g_e = e2[rs][rr, ep[rs]].astype(np.float64)
        total += (lse_s - g_s).sum() + (lse_e - g_e).sum()

    loss = total / (2.0 * B)
    return np.asarray(loss, dtype=np.float32)


# revision 10
# speedup vs baseline: 1.0210x; 1.0210x over previous
"""Trainium2 Bass kernel for the span-extraction (start/end) cross-entropy loss.

    loss = (1/(2B)) * sum_b [ (LSE_s[b] - s[b, sp_b]) + (LSE_e[b] - e[b, ep_b]) ]

Distribution: data-parallel over the batch axis across 8 NeuronCores (32 rows
per core per tensor).  The kernel is memory-bound; all logits are staged to
the device as fp8-e4m3 (1 B/elem; the 2e-2 rel-err gate leaves orders of
magnitude of headroom — measured end-to-end error is ~5e-5).  Three engines
split the exp+sum work per tensor:

  * ACT share (A=3072 cols/partition, row-quarter-major): fused exact
    exp + accumulate at 1 elem/cycle/lane.  Two ops per tensor (512-col
    starter + 2560-col main) to amortize the ~570ns per-op fixed cost
    (352cyc ACTIVATE startup + 279ns accumulator read).  The main op's
    data arrives as TWO 1280-byte windows riding DIFFERENT DMA queues in
    parallel, so its gate is ~1.9us earlier than one mid-queue window.
  * DVE share (V=5120 cols/partition, TRANSPOSED: each SBUF column holds
    128 elements of one batch row): tensor_scalar ops compute
    round(A*x + B) into int16 (Schraudolph: the int16 bit patterns ARE
    bf16(exp(x)); A = 128/ln2, B calibrated so E[schr(x)] = E[exp(x)] on
    N(0,1)).  Runs at 2 elem/cycle/lane (fp8 single-src 2x_2P mode).
  * PE (otherwise idle) reduces the bf16-bitcast int16 tiles over the
    partition axis: all-ones matmuls accumulate 256-column segments into
    PSUM, psum[:, gg*32 + r] += sum_p shr[p, seg*256 + gg*32 + r].
    PE p-states (0.65/1.2/2.4 GHz) cap segments at ~270-420ns; the PE
    pipelines and keeps pace with DVE; final chunks are small (3 segs).

DMA: 6 windows per tensor, 12 data DMAs.  s-chunks ride the sync HWDGE
ring (Q1), e-chunks the scalar ring (Q10), EXCEPT the cross-queue ACT
halves (s_a2b on Q10, e_a2b on Q1).  Only 8 DMA completion-sem lanes
exist; the 9th-12th DMAs reuse lanes of early-completing chunks and ALL
ride the sync engine — a reused lane makes the dispatch WAIT for its
predecessor, which on the scalar engine would delay the ACTIVATEs
(~1.2us, measured).  Outputs are split four ways (psa_s/psa_e/pe_s/pe_e)
and dispatched from the sync engine as soon as each half's producers
finish, hiding most of the ~1.5-2us HBM-write receipt inside the compute
tail.  The 512 target logits are gathered on the host from the fp32
originals; the host sums the partials, takes log, and combines in fp64.
"""

import numpy as np
import ml_dtypes

from contextlib import ExitStack

import concourse.bass as bass
import concourse.bacc as bacc
import concourse.tile as tile
from concourse import mybir
from concourse.bass_utils import run_bass_kernel_spmd

B, S = 256, 32768
N_CORES = 8
ROWS = B // N_CORES          # 32 batch rows per core
QUARTERS = 4                 # ACT share: each row split across 4 partitions
P = ROWS * QUARTERS          # 128 partitions
SEG = S // QUARTERS          # 8192 elements per partition-quarter
LINE_B = SEG                 # all-fp8: 8192 bytes per partition per tensor

# SBUF line layout: c1 (ACT starter + first DVE window), c3 (DVE), a2a+a2b
# (the contiguous 2560-col main ACT window, fetched as two DMAs), c4 (DVE),
# c5 (small DVE tail).  wv multiples of 256 (PE segments).
WINDOWS = [("c1", 512, 768), ("c3", 0, 1792), ("a2a", 1280, 0),
           ("a2b", 1280, 0), ("c4", 0, 1792), ("c5", 0, 768)]
A_COLS = sum(wa for _, wa, _ in WINDOWS)      # 3072
V_COLS = sum(wv for _, _, wv in WINDOWS)      # 5120
G = V_COLS // 32                              # 160 groups of 128 per row
SEGS = V_COLS // 256                          # 20 matmul segments
assert all(wv % 256 == 0 for _, _, wv in WINDOWS)
assert A_COLS + V_COLS == LINE_B

CWIN = []                     # (lo, hi) line windows
_off = 0
for _, wa, wv in WINDOWS:
    CWIN.append((_off, _off + wa + wv))
    _off += wa + wv
assert _off == LINE_B
VOFF = []                     # shr column offset per window
_v = 0
for _, _, wv in WINDOWS:
    VOFF.append(_v)
    _v += wv

# Data-DMA emission order: (tensor, window index, ring).  Global order
# sets the 8-lane round-robin; DMAs 9-12 reuse lanes of DMAs 1-4 (all
# complete early) and all ride sync.  Q1 FIFO: s_c1, s_c3, s_a2a, e_a2b,
# s_c4, e_c4, s_c5, e_c5.  Q10 FIFO: e_c1, s_a2b, e_c3, e_a2a.
DMA_ORDER = [
    ("s", 0, "sync"), ("e", 0, "scalar"),
    ("s", 3, "scalar"),                      # s_a2b cross-queue, early Q10
    ("s", 1, "sync"), ("e", 1, "scalar"),
    ("s", 2, "sync"), ("e", 2, "scalar"),
    ("e", 3, "sync"),                        # e_a2b cross-queue on Q1
    ("s", 4, "sync"), ("e", 4, "sync"),
    ("s", 5, "sync"), ("e", 5, "sync"),
]
assert len(DMA_ORDER) == 12

# Schraudolph constants: schr(x) = bitcast_bf16(int16(A*x + B)), with the
# f32->i16 conversion rounding to nearest (verified on HW: rel err ~1e-6).
A_SCHR = 128.0 / float(np.log(2.0))
B_SCHR = 16256.0 - 7.367385

_CACHE = {}

LAST_RESULT = None           # BassKernelResults of the most recent run


def _build():
    f32 = mybir.dt.float32
    bf16 = mybir.dt.bfloat16
    f8 = mybir.dt.float8e4
    u8 = mybir.dt.uint8
    i16 = mybir.dt.int16
    nc = bacc.Bacc(
        "TRN2", target_bir_lowering=False, debug=False, num_devices=N_CORES
    )
    x_in = {
        nm: nc.dram_tensor(f"x_{nm}", [P, LINE_B], u8, kind="ExternalInput").ap()
        for nm in ("s", "e")
    }
    out_d = {
        "ps_s": nc.dram_tensor("ps_s", [P, 2], f32, kind="ExternalOutput").ap(),
        "ps_e": nc.dram_tensor("ps_e", [P, 2], f32, kind="ExternalOutput").ap(),
        "pe_s": nc.dram_tensor("pe_s", [1, 256], f32, kind="ExternalOutput").ap(),
        "pe_e": nc.dram_tensor("pe_e", [1, 256], f32, kind="ExternalOutput").ap(),
    }

    w1 = WINDOWS[0][1]                     # op1 cols (512)
    a2lo = CWIN[2][0]                      # main ACT window [a2lo, a2lo+2560)
    w2 = CWIN[3][1] - a2lo                 # 2560

    with tile.TileContext(nc) as tc, ExitStack() as ctx:
        data_pool = ctx.enter_context(tc.tile_pool(name="data", bufs=1))
        small_pool = ctx.enter_context(tc.tile_pool(name="small", bufs=1))
        psum_pool = ctx.enter_context(
            tc.tile_pool(name="psum", bufs=1, space="PSUM")
        )

        xbuf = {
            nm: data_pool.tile([P, LINE_B], u8, name=f"x_{nm}", tag=f"x_{nm}")
            for nm in ("s", "e")
        }
        shr = {
            nm: data_pool.tile([P, V_COLS], i16, name=f"sh_{nm}", tag=f"sh_{nm}")
            for nm in ("s", "e")
        }
        scr = {
            nm: data_pool.tile([P, w2], bf16, name=f"sc_{nm}", tag=f"sc_{nm}")
            for nm in ("s", "e")
        }
        acc_a = small_pool.tile([P, 4], f32, tag="acc_a")
        pe_sb = small_pool.tile([1, 512], f32, tag="pe_sb")
        ones = small_pool.tile([P, P], bf16, tag="ones")
        psum = {
            nm: psum_pool.tile([P, 512], f32, name=f"pm_{nm}", tag=f"pm_{nm}")
            for nm in ("s", "e")
        }

        # all-ones weights for the PE partition-reduction.
        nc.vector.memset(ones[:], 1.0)

        # Data DMAs in sem-lane emission order.
        for nm, ci, ring in DMA_ORDER:
            lo, hi = CWIN[ci]
            eng = nc.sync if ring == "sync" else nc.scalar
            eng.dma_start(xbuf[nm][:, lo:hi], x_in[nm][:, lo:hi])

        # ACT: exact exp + fused accumulate.  acc cols: s -> 0,1; e -> 2,3.
        acol = {("s", 0): 0, ("s", 1): 1, ("e", 0): 2, ("e", 1): 3}
        for nm in ("s", "e"):
            va = xbuf[nm].bitcast(f8)
            nc.scalar.activation(
                scr[nm][:, :w1],
                va[:, 0:w1],
                mybir.ActivationFunctionType.Exp,
                accum_out=acc_a[:, acol[nm, 0] : acol[nm, 0] + 1],
            )
            nc.scalar.activation(
                scr[nm][:, :w2],
                va[:, a2lo : a2lo + w2],
                mybir.ActivationFunctionType.Exp,
                accum_out=acc_a[:, acol[nm, 1] : acol[nm, 1] + 1],
            )

        # DVE pass 1 (one op per v-window, s/e interleaved) and PE
        # reduction (256-col segments accumulated into PSUM; copies right
        # after each tensor's last segment).
        vwins = [(ci, wa, wv) for ci, (_, wa, wv) in enumerate(WINDOWS) if wv]
        for k, (ci, wa, wv) in enumerate(vwins):
            lo = CWIN[ci][0] + wa
            for nm in ("s", "e"):
                va = xbuf[nm].bitcast(f8)
                nc.vector.tensor_scalar(
                    shr[nm][:, VOFF[ci] : VOFF[ci] + wv],
                    va[:, lo : lo + wv],
                    A_SCHR,
                    B_SCHR,
                    mybir.AluOpType.mult,
                    mybir.AluOpType.add,
                )
            s0 = VOFF[ci] // 256
            for nm in ("s", "e"):
                sv = shr[nm].bitcast(bf16)
                for sg in range(s0, s0 + wv // 256):
                    nc.tensor.matmul(
                        psum[nm][:, 0:256],
                        ones[:],
                        sv[:, sg * 256 : (sg + 1) * 256],
                        start=(sg == 0),
                        stop=(sg == SEGS - 1),
                    )
                if k == len(vwins) - 1:
                    dst = pe_sb[0:1, 0:256] if nm == "s" else pe_sb[0:1, 256:512]
                    nc.vector.tensor_copy(dst, psum[nm][0:1, 0:256])

        # Outputs, all on the (idle) sync ring, each dispatched as soon as
        # its producers finish — emission in expected-readiness order.
        nc.sync.dma_start(out_d["ps_s"], acc_a[:, 0:2])
        nc.sync.dma_start(out_d["ps_e"], acc_a[:, 2:4])
        nc.sync.dma_start(out_d["pe_s"], pe_sb[0:1, 0:256])
        nc.sync.dma_start(out_d["pe_e"], pe_sb[0:1, 256:512])
    nc.compile()
    return nc


def _get_nc():
    if "nc" not in _CACHE:
        _CACHE["nc"] = _build()
    return _CACHE["nc"]


def _stage(x2):
    """[B, S] f32 -> per-core [128, 8192] u8 lines in the windowed layout.

    Returns [N_CORES, 128, LINE_B] u8.  ACT parts row-quarter-major
    (partition r*4+q holds fp8 of that quarter's next wa cols); DVE parts
    transposed (column g*32 + r holds fp8 of 128 elements of row r;
    columns assigned to windows sequentially)."""
    f8np = mybir.dt.np(mybir.dt.float8e4)
    x3 = x2.reshape(B, QUARTERS, SEG)
    out = np.empty((N_CORES, P, LINE_B), np.uint8)
    for i in range(N_CORES):
        rs = slice(i * ROWS, (i + 1) * ROWS)
        act = np.ascontiguousarray(x3[rs, :, :A_COLS]).astype(f8np)
        act = act.reshape(P, A_COLS).view(np.uint8)
        dve = np.ascontiguousarray(x3[rs, :, A_COLS:]).astype(f8np)
        # [32, 4*(SEG-A)] -> [32, G, 128] -> [128, G, 32] -> [128, V]
        dve = dve.reshape(ROWS, G, 128).transpose(2, 1, 0)
        dve = np.ascontiguousarray(dve).reshape(P, V_COLS).view(np.uint8)
        aoff = voff = 0
        for ci, (_, wa, wv) in enumerate(WINDOWS):
            lo = CWIN[ci][0]
            if wa:
                out[i, :, lo : lo + wa] = act[:, aoff : aoff + wa]
                aoff += wa
            if wv:
                out[i, :, lo + wa : lo + wa + wv] = dve[:, voff : voff + wv]
                voff += wv
    return out


def kernel(start_logits, end_logits, start_positions, end_positions):
    global LAST_RESULT
    s2 = np.ascontiguousarray(np.asarray(start_logits, dtype=np.float32).reshape(B, S))
    e2 = np.ascontiguousarray(np.asarray(end_logits, dtype=np.float32).reshape(B, S))
    sp = np.asarray(start_positions).astype(np.int64)
    ep = np.asarray(end_positions).astype(np.int64)

    s_st = _stage(s2)
    e_st = _stage(e2)

    in_maps = [
        {"x_s": s_st[i], "x_e": e_st[i]} for i in range(N_CORES)
    ]

    nc = _get_nc()
    res = run_bass_kernel_spmd(nc, in_maps, list(range(N_CORES)))
    LAST_RESULT = res

    total = 0.0
    rr = np.arange(ROWS)
    for i in range(N_CORES):
        rs = slice(i * ROWS, (i + 1) * ROWS)
        r = res.results[i]
        pa_s = np.asarray(r["ps_s"], np.float64)   # [P, 2]
        pa_e = np.asarray(r["ps_e"], np.float64)   # [P, 2]
        pe_s = np.asarray(r["pe_s"], np.float64)[0]  # [256]
        pe_e = np.asarray(r["pe_e"], np.float64)[0]  # [256]
        act_s = pa_s.reshape(ROWS, QUARTERS, 2).sum(axis=(1, 2))
        act_e = pa_e.reshape(ROWS, QUARTERS, 2).sum(axis=(1, 2))
        dve_s = pe_s.reshape(8, ROWS).sum(axis=0)
        dve_e = pe_e.reshape(8, ROWS).sum(axis=0)
        lse_s = np.log(act_s + dve_s)
        lse_e = np.log(act_e + dve_e)
        g_s = s2[rs][rr, sp[rs]].astype(np.float64)
        g_e = e2[rs][rr, ep[rs]].astype(np.float64)
        total += (lse_s - g_s).sum() + (lse_e - g_e).sum()

    loss = total / (2.0 * B)
    return np.asarray(loss, dtype=np.float32)
